# revision 21
# baseline (speedup 1.0000x reference)
"""Multi-head self-attention (RoPE, causal) Trainium2 Bass kernel, 8 NeuronCores.

Sharding: data-parallel over batch (B=2) x tensor-parallel over heads
(16 heads -> 4 groups of 4). Core c handles batch b=c//4, heads 4*(c%4)..4*(c%4)+3.
Each core computes its 4 heads' attention plus a partial output projection;
the host sums the 4 partial outputs per batch element.

Device-side layout (per core):
  x^T [1024d, L] bf16 (host pre-transposed). Q^T/K^T [256c, L] = W_slice @ x^T.
  RoPE applied in-place on the [channel, L] layout with a partition pair-swap
  (DVE stream_shuffle) + cos / signed-sin tables: 4 DVE ops per tile.
  Attention in transposed [k, q] layout: T = K^T.T @ Q^T (K=64 contraction per
  head), P^T = exp(T/8) (no max subtraction; scores are O(1) by construction),
  O'^T and softmax denominators (ones-matmul) accumulated in PSUM over k tiles.
  Causal: above-diagonal k-tiles skipped entirely, diagonal ones masked.
  Row-group-tiled matmuls must land in distinct PSUM banks, and every open
  PSUM accumulation group owns its whole (partition-range x bank) zero-region.
  Output projection per q chunk from A^T tiles (stationary) x Wo^T slices;
  host sums the 4 partial projections per batch element.
"""
import sys, math

sys.path.insert(0, "/opt/trn_rl_repo")

import numpy as np
import ml_dtypes

import concourse.bacc as bacc
import concourse.bass as bass
import concourse.mybir as mybir
import concourse.tile as tile
from concourse.bass_utils import run_bass_kernel_spmd

BF16 = mybir.dt.bfloat16
F32 = mybir.dt.float32
NPBF16 = ml_dtypes.bfloat16

D_MODEL = 1024
D_HEAD = 64
HALF = D_HEAD // 2
ROPE_THETA = 10000.0
N_CORES = 8
C = 256  # channels per core (4 heads x 64)
SWAP32 = [i ^ 1 for i in range(32)]


def _body(nc, tc, L, pp, rtp, ptp, rip, osp):
    n_lt = L // 128
    n_qk = max(1, L // 512)
    qkw = min(512, L)
    qw = min(512, L)
    n_qch = L // qw
    n_msk = qw // 128

    xt_d = nc.dram_tensor("xt", [D_MODEL, L], BF16, kind="ExternalInput").ap()
    wq_d = nc.dram_tensor("wqt", [D_MODEL, C], BF16, kind="ExternalInput").ap()
    wk_d = nc.dram_tensor("wkt", [D_MODEL, C], BF16, kind="ExternalInput").ap()
    wv_d = nc.dram_tensor("wvt", [D_MODEL, C], BF16, kind="ExternalInput").ap()
    wo_d = nc.dram_tensor("wot", [C, D_MODEL], BF16, kind="ExternalInput").ap()
    cos_d = nc.dram_tensor("cosb", [128, L], F32, kind="ExternalInput").ap()
    sin_d = nc.dram_tensor("ssin", [128, L], F32, kind="ExternalInput").ap()
    mk_d = nc.dram_tensor("masks", [128, n_msk * qw], BF16,
                          kind="ExternalInput").ap()
    out_d = nc.dram_tensor("out", [L, D_MODEL], F32, kind="ExternalOutput").ap()

    # ---- persistent SBUF tensors
    wq = pp.tile([128, 8, C], BF16)
    wk = pp.tile([128, 8, C], BF16)
    wv = pp.tile([128, 8, C], BF16)
    wo = pp.tile([128, 2, D_MODEL], BF16)
    cs = pp.tile([128, L], F32)
    sn = pp.tile([128, L], F32)
    mks = pp.tile([128, n_msk * qw], BF16)
    ones = pp.tile([128, 64], BF16)
    qt = pp.tile([128, 2, L], BF16)
    kt_ = pp.tile([128, 2, L], BF16)
    vt = pp.tile([128, n_lt, C], BF16)
    at = pp.tile([128, 2, L], BF16)
    xts = [pp.tile([128, L], BF16, name=f"xt{i}") for i in range(8)]

    # ---- loads (small tensors first; x^T split per d-tile for pipelining)
    nc.sync.dma_start(out=wq[:], in_=wq_d.rearrange("(a p) c -> p a c", p=128))
    nc.sync.dma_start(out=wk[:], in_=wk_d.rearrange("(a p) c -> p a c", p=128))
    nc.sync.dma_start(out=cs[:], in_=cos_d)
    nc.sync.dma_start(out=sn[:], in_=sin_d)
    nc.sync.dma_start(out=wv[:], in_=wv_d.rearrange("(a p) c -> p a c", p=128))
    # x^T arrives in (L-chunk, d-tile) pieces matching QKV consumption order
    for lc in range(0, L, 512):
        w_ = min(512, L - lc)
        for i in range(8):
            nc.sync.dma_start(out=xts[i][:, lc:lc + w_],
                              in_=xt_d[i * 128:(i + 1) * 128, lc:lc + w_])
    nc.sync.dma_start(out=wo[:], in_=wo_d.rearrange("(a p) e -> p a e", p=128))
    nc.sync.dma_start(out=mks[:], in_=mk_d)
    nc.gpsimd.memset(ones[:], 1.0)

    # ---- Q^T / K^T projection + RoPE (shuffle pair-swap + cos/signed-sin)
    with tc.tile_pool(name="qk_ps", bufs=6, space="PSUM") as qkps, \
         tc.tile_pool(name="v_ps", bufs=2, space="PSUM") as vps:
        for qc in range(n_qk):
            ls = qc * qkw
            ps = {}
            for nm, w in (("q", wq), ("k", wk)):
                for ct in (0, 1):
                    p = qkps.tile([128, qkw], F32, tag="qkps",
                                  name=f"ps_{nm}{ct}_{qc}")
                    for dt_ in range(8):
                        nc.tensor.matmul(
                            p[:],
                            lhsT=w[:, dt_, ct * 128:ct * 128 + 128],
                            rhs=xts[dt_][:, ls:ls + qkw],
                            start=(dt_ == 0), stop=(dt_ == 7))
                    ps[(nm, ct)] = p
            for nm, dst in (("q", qt), ("k", kt_)):
                for ct in (0, 1):
                    p = ps[(nm, ct)]
                    sh = rtp.tile([128, qkw], F32, tag="t",
                                  name=f"sh_{nm}{ct}{qc}")
                    t1 = rtp.tile([128, qkw], F32, tag="t",
                                  name=f"t1_{nm}{ct}{qc}")
                    t2 = rtp.tile([128, qkw], F32, tag="t",
                                  name=f"t2_{nm}{ct}{qc}")
                    nc.vector.stream_shuffle(sh[:], p[:], SWAP32)
                    nc.vector.tensor_mul(t1[:], p[:], cs[:, ls:ls + qkw])
                    nc.gpsimd.tensor_mul(t2[:], sh[:], sn[:, ls:ls + qkw])
                    nc.gpsimd.tensor_add(dst[:, ct, ls:ls + qkw], t1[:], t2[:])
            # V projection for this chunk's L tiles (keeps attention startable)
            for lt in range(ls // 128, (ls + qkw) // 128):
                pv = vps.tile([128, C], F32, tag="vps", name=f"pv_{lt}")
                for dt_ in range(8):
                    nc.tensor.matmul(
                        pv[:],
                        lhsT=xts[dt_][:, lt * 128:lt * 128 + 128],
                        rhs=wv[:, dt_, :],
                        start=(dt_ == 0), stop=(dt_ == 7))
                nc.scalar.copy(vt[:, lt, :], pv[:])


    # ---- attention + interleaved output projection, per 512-wide q chunk
    scale = 1.0 / math.sqrt(D_HEAD)
    with tc.tile_pool(name="att_ps", bufs=2, space="PSUM") as atps, \
         tc.tile_pool(name="o_ps", bufs=2, space="PSUM") as ops_, \
         tc.tile_pool(name="r_ps", bufs=2, space="PSUM") as rps:
        for qc in range(n_qch):
            qs = qc * qw
            ktmax = (qs + qw) // 128
            for pair in range(2):
                po = ops_.tile([128, qw], F32, tag="o", name=f"po_{pair}_{qc}")
                pr = rps.tile([128, qw], F32, tag="r", name=f"pr_{pair}_{qc}")
                for kt in range(ktmax):
                    pt_ps = atps.tile([128, 1024], F32, tag="tps",
                                      name=f"pt_{pair}_{qc}_{kt}")
                    for hloc in range(2):
                        # K=64 contraction; row groups 0-1 / 2-3 concurrent;
                        # each head's T block in its own PSUM bank
                        nc.tensor.matmul(
                            pt_ps[:, 512 * hloc:512 * hloc + qw],
                            lhsT=kt_[64 * hloc:64 * hloc + 64, pair,
                                     kt * 128:kt * 128 + 128],
                            rhs=qt[64 * hloc:64 * hloc + 64, pair, qs:qs + qw],
                            start=True, stop=True,
                            tile_position=(64 * hloc, 0),
                            skip_group_check=True)
                    pt_sb = ptp.tile([128, 1024], BF16, tag="p",
                                     name=f"ptsb_{pair}_{qc}_{kt}")
                    if qw == 512:
                        nc.scalar.activation(pt_sb[:], pt_ps[:],
                                             mybir.ActivationFunctionType.Exp,
                                             scale=scale)
                    else:
                        for hloc in range(2):
                            nc.scalar.activation(
                                pt_sb[:, 512 * hloc:512 * hloc + qw],
                                pt_ps[:, 512 * hloc:512 * hloc + qw],
                                mybir.ActivationFunctionType.Exp, scale=scale)
                    off = kt * 128 - qs
                    if off >= 0:
                        m = off // 128
                        for hloc in range(2):
                            nc.vector.tensor_mul(
                                pt_sb[:, 512 * hloc:512 * hloc + qw],
                                pt_sb[:, 512 * hloc:512 * hloc + qw],
                                mks[:, m * qw:m * qw + qw])
                    for hloc in range(2):
                        h = 2 * pair + hloc
                        nc.tensor.matmul(
                            po[64 * hloc:64 * hloc + 64, :],
                            lhsT=vt[:, kt, 64 * h:64 * h + 64],
                            rhs=pt_sb[:, 512 * hloc:512 * hloc + qw],
                            start=(kt == 0), stop=(kt == ktmax - 1),
                            tile_position=(0, 64 * hloc),
                            skip_group_check=True)
                    for hloc in range(2):
                        nc.tensor.matmul(
                            pr[64 * hloc:64 * hloc + 64, :],
                            lhsT=ones[:, 0:64],
                            rhs=pt_sb[:, 512 * hloc:512 * hloc + qw],
                            start=(kt == 0), stop=(kt == ktmax - 1),
                            tile_position=(0, 64 * hloc),
                            skip_group_check=True)
                ri = rip.tile([128, 512], F32, tag="ri",
                              name=f"ri_{pair}_{qc}")
                nc.vector.reciprocal_approx_fast(out=ri[:, 0:qw], in_=pr[:])
                nc.vector.tensor_mul(at[:, pair, qs:qs + qw], po[:],
                                     ri[:, 0:qw])
    # ---- output projection (separate phase; attention keeps 8 psum banks)
    with tc.tile_pool(name="out_ps", bufs=2, space="PSUM") as outps:
        for qtl in range(n_lt):
            pout = outps.tile([128, 1024], F32, tag="outps",
                              name=f"pout_{qtl}")
            for ct in range(2):
                for eh in range(2):
                    nc.tensor.matmul(
                        pout[:, eh * 512:eh * 512 + 512],
                        lhsT=at[:, ct, qtl * 128:qtl * 128 + 128],
                        rhs=wo[:, ct, eh * 512:eh * 512 + 512],
                        start=(ct == 0), stop=(ct == 1),
                        skip_group_check=True)
            stg = osp.tile([128, 1024], F32, tag="stg", name=f"stg_{qtl}")
            nc.vector.tensor_copy(stg[:, 0:512], pout[:, 0:512])
            nc.scalar.copy(stg[:, 512:1024], pout[:, 512:1024])
            nc.sync.dma_start(out=out_d[qtl * 128:qtl * 128 + 128, :],
                              in_=stg[:])


def build_nc(L=2048):
    """Build + compile the per-core Bass program (same NEFF on all 8 cores)."""
    assert L % 256 == 0
    nc = bacc.Bacc("TRN2", target_bir_lowering=False, debug=False,
                   num_devices=N_CORES)
    with tile.TileContext(nc) as tc:
        with tc.tile_pool(name="persist", bufs=1) as pp, \
             tc.tile_pool(name="ropet", bufs=6) as rtp, \
             tc.tile_pool(name="ptp", bufs=3) as ptp, \
             tc.tile_pool(name="rinvp", bufs=2) as rip, \
             tc.tile_pool(name="ostg", bufs=3) as osp:
            _body(nc, tc, L, pp, rtp, ptp, rip, osp)
    nc.compile()
    return nc


_NC_CACHE = {}


def _get_nc(L):
    if L not in _NC_CACHE:
        _NC_CACHE[L] = build_nc(L)
    return _NC_CACHE[L]


def make_inputs(x, token_positions, Wq, Wk, Wv, Wo):
    """Host-side shard/layout prep -> list of 8 per-core input dicts."""
    B, L, _ = x.shape
    pos = np.asarray(token_positions).astype(np.float64)
    S = ROPE_THETA ** (-2.0 / D_HEAD)
    thetas = S ** np.arange(HALF, dtype=np.float64)
    ang = pos[:, None] * thetas[None, :]          # [L, 32]
    cosL = np.cos(ang).T                          # [32, L]
    sinL = np.sin(ang).T
    # per-channel tables on the natural (head, dim) layout:
    # row p (within a 64-row head block): pair i = (p%64)//2
    # cosb[p] = cos(theta_i * pos); ssin[p] = -sin if dim even else +sin
    cosb = np.empty((128, L), dtype=np.float64)
    ssin = np.empty((128, L), dtype=np.float64)
    for p in range(128):
        i = (p % 64) // 2
        cosb[p] = cosL[i]
        ssin[p] = -sinL[i] if (p % 2 == 0) else sinL[i]
    cosb = cosb.astype(np.float32)
    ssin = ssin.astype(np.float32)

    qw = min(512, L)
    r = np.arange(128)[:, None]
    col = np.arange(qw)[None, :]
    masks = np.concatenate(
        [(col >= r + 128 * m) for m in range(qw // 128)],
        axis=1).astype(NPBF16)  # [128, (qw//128)*qw]

    xts = [np.ascontiguousarray(x[b].astype(NPBF16).T) for b in range(B)]
    in_maps = []
    shard_cache = {}
    for core in range(N_CORES):
        b, hg = core // 4, core % 4
        if hg not in shard_cache:
            rows = slice(hg * 256, hg * 256 + 256)
            shard_cache[hg] = {
                "wqt": np.ascontiguousarray(Wq[rows].astype(NPBF16).T),
                "wkt": np.ascontiguousarray(Wk[rows].astype(NPBF16).T),
                "wvt": np.ascontiguousarray(Wv[rows].astype(NPBF16).T),
                "wot": np.ascontiguousarray(Wo[:, rows].astype(NPBF16).T),
            }
        m = dict(shard_cache[hg])
        m["xt"] = xts[b]
        m["cosb"] = cosb
        m["ssin"] = ssin
        m["masks"] = masks
        in_maps.append(m)
    return in_maps


def kernel(x, token_positions, Wq, Wk, Wv, Wo):
    x = np.asarray(x); Wq = np.asarray(Wq); Wk = np.asarray(Wk)
    Wv = np.asarray(Wv); Wo = np.asarray(Wo)
    B, L, _ = x.shape
    nc = _get_nc(L)
    in_maps = make_inputs(x, token_positions, Wq, Wk, Wv, Wo)
    res = run_bass_kernel_spmd(nc, in_maps, core_ids=list(range(N_CORES)))
    out = np.zeros((B, L, D_MODEL), dtype=np.float32)
    for core in range(N_CORES):
        out[core // 4] += res.results[core]["out"]
    return out


# revision 22
# speedup vs baseline: 1.0133x; 1.0133x over previous
"""Multi-head self-attention (RoPE, causal) Trainium2 Bass kernel, 8 NeuronCores.

Sharding: data-parallel over batch (B=2) x tensor-parallel over heads
(16 heads -> 4 groups of 4). Core c handles batch b=c//4, heads 4*(c%4)..4*(c%4)+3.
Each core computes its 4 heads' attention plus a partial output projection;
the host sums the 4 partial outputs per batch element.

Device-side layout (per core):
  x^T [1024d, L] bf16 (host pre-transposed). Q^T/K^T [256c, L] = W_slice @ x^T.
  RoPE applied in-place on the [channel, L] layout with a partition pair-swap
  (DVE stream_shuffle) + cos / signed-sin tables: 4 DVE ops per tile.
  Attention in transposed [k, q] layout: T = K^T.T @ Q^T (K=64 contraction per
  head), P^T = exp(T/8) (no max subtraction; scores are O(1) by construction),
  O'^T and softmax denominators (ones-matmul) accumulated in PSUM over k tiles.
  Causal: above-diagonal k-tiles skipped entirely, diagonal ones masked.
  Row-group-tiled matmuls must land in distinct PSUM banks, and every open
  PSUM accumulation group owns its whole (partition-range x bank) zero-region.
  Output projection per q chunk from A^T tiles (stationary) x Wo^T slices;
  host sums the 4 partial projections per batch element.
"""
import sys, math

sys.path.insert(0, "/opt/trn_rl_repo")

import numpy as np
import ml_dtypes

import concourse.bacc as bacc
import concourse.bass as bass
import concourse.mybir as mybir
import concourse.tile as tile
from concourse.bass_utils import run_bass_kernel_spmd

BF16 = mybir.dt.bfloat16
F32 = mybir.dt.float32
NPBF16 = ml_dtypes.bfloat16

D_MODEL = 1024
D_HEAD = 64
HALF = D_HEAD // 2
ROPE_THETA = 10000.0
N_CORES = 8
C = 256  # channels per core (4 heads x 64)
SWAP32 = [i ^ 1 for i in range(32)]


def _body(nc, tc, L, pp, rtp, ptp, rip, osp):
    n_lt = L // 128
    n_qk = max(1, L // 512)
    qkw = min(512, L)
    qw = min(512, L)
    n_qch = L // qw
    n_msk = qw // 128

    xt_d = nc.dram_tensor("xt", [D_MODEL, L], BF16, kind="ExternalInput").ap()
    wq_d = nc.dram_tensor("wqt", [D_MODEL, C], BF16, kind="ExternalInput").ap()
    wk_d = nc.dram_tensor("wkt", [D_MODEL, C], BF16, kind="ExternalInput").ap()
    wv_d = nc.dram_tensor("wvt", [D_MODEL, C], BF16, kind="ExternalInput").ap()
    wo_d = nc.dram_tensor("wot", [C, D_MODEL], BF16, kind="ExternalInput").ap()
    cos_d = nc.dram_tensor("cosb", [128, L], F32, kind="ExternalInput").ap()
    sin_d = nc.dram_tensor("ssin", [128, L], F32, kind="ExternalInput").ap()
    mk_d = nc.dram_tensor("masks", [128, n_msk * qw], BF16,
                          kind="ExternalInput").ap()
    out_d = nc.dram_tensor("out", [L, D_MODEL], F32, kind="ExternalOutput").ap()

    # ---- persistent SBUF tensors
    wq = pp.tile([128, 8, C], BF16)
    wk = pp.tile([128, 8, C], BF16)
    wv = pp.tile([128, 8, C], BF16)
    wo = pp.tile([128, 2, D_MODEL], BF16)
    cs = pp.tile([128, L], F32)
    sn = pp.tile([128, L], F32)
    mks = pp.tile([128, n_msk * qw], BF16)
    ones = pp.tile([128, 64], BF16)
    qt = pp.tile([128, 2, L], BF16)
    kt_ = pp.tile([128, 2, L], BF16)
    vt = pp.tile([128, n_lt, C], BF16)
    at = pp.tile([128, 2, L], BF16)
    xts = [pp.tile([128, L], BF16, name=f"xt{i}") for i in range(8)]

    # ---- loads (small tensors first; x^T split per d-tile for pipelining)
    nc.sync.dma_start(out=wq[:], in_=wq_d.rearrange("(a p) c -> p a c", p=128))
    nc.sync.dma_start(out=wk[:], in_=wk_d.rearrange("(a p) c -> p a c", p=128))
    nc.sync.dma_start(out=cs[:], in_=cos_d)
    nc.sync.dma_start(out=sn[:], in_=sin_d)
    nc.sync.dma_start(out=wv[:], in_=wv_d.rearrange("(a p) c -> p a c", p=128))
    # x^T arrives in (L-chunk, d-tile) pieces matching QKV consumption order
    for lc in range(0, L, 512):
        w_ = min(512, L - lc)
        for i in range(8):
            nc.sync.dma_start(out=xts[i][:, lc:lc + w_],
                              in_=xt_d[i * 128:(i + 1) * 128, lc:lc + w_])
    nc.sync.dma_start(out=wo[:], in_=wo_d.rearrange("(a p) e -> p a e", p=128))
    nc.sync.dma_start(out=mks[:], in_=mk_d)
    nc.gpsimd.memset(ones[:], 1.0)

    # ---- Q^T / K^T projection + RoPE (shuffle pair-swap + cos/signed-sin)
    with tc.tile_pool(name="qk_ps", bufs=6, space="PSUM") as qkps, \
         tc.tile_pool(name="v_ps", bufs=2, space="PSUM") as vps:
        for qc in range(n_qk):
            ls = qc * qkw
            ps = {}
            for nm, w in (("q", wq), ("k", wk)):
                for ct in (0, 1):
                    p = qkps.tile([128, qkw], F32, tag="qkps",
                                  name=f"ps_{nm}{ct}_{qc}")
                    for dt_ in range(8):
                        nc.tensor.matmul(
                            p[:],
                            lhsT=w[:, dt_, ct * 128:ct * 128 + 128],
                            rhs=xts[dt_][:, ls:ls + qkw],
                            start=(dt_ == 0), stop=(dt_ == 7))
                    ps[(nm, ct)] = p
            for nm, dst in (("q", qt), ("k", kt_)):
                for ct in (0, 1):
                    p = ps[(nm, ct)]
                    sh = rtp.tile([128, qkw], F32, tag="t",
                                  name=f"sh_{nm}{ct}{qc}")
                    t1 = rtp.tile([128, qkw], F32, tag="t",
                                  name=f"t1_{nm}{ct}{qc}")
                    t2 = rtp.tile([128, qkw], F32, tag="t",
                                  name=f"t2_{nm}{ct}{qc}")
                    nc.vector.stream_shuffle(sh[:], p[:], SWAP32)
                    nc.vector.tensor_mul(t1[:], p[:], cs[:, ls:ls + qkw])
                    nc.vector.tensor_mul(t2[:], sh[:], sn[:, ls:ls + qkw])
                    nc.vector.tensor_add(dst[:, ct, ls:ls + qkw], t1[:], t2[:])
            # V projection for this chunk's L tiles (keeps attention startable)
            for lt in range(ls // 128, (ls + qkw) // 128):
                pv = vps.tile([128, C], F32, tag="vps", name=f"pv_{lt}")
                for dt_ in range(8):
                    nc.tensor.matmul(
                        pv[:],
                        lhsT=xts[dt_][:, lt * 128:lt * 128 + 128],
                        rhs=wv[:, dt_, :],
                        start=(dt_ == 0), stop=(dt_ == 7))
                nc.scalar.copy(vt[:, lt, :], pv[:])


    # ---- attention + interleaved output projection, per 512-wide q chunk
    scale = 1.0 / math.sqrt(D_HEAD)
    with tc.tile_pool(name="att_ps", bufs=2, space="PSUM") as atps, \
         tc.tile_pool(name="o_ps", bufs=2, space="PSUM") as ops_, \
         tc.tile_pool(name="r_ps", bufs=2, space="PSUM") as rps:
        for qc in range(n_qch):
            qs = qc * qw
            ktmax = (qs + qw) // 128
            for pair in range(2):
                po = ops_.tile([128, qw], F32, tag="o", name=f"po_{pair}_{qc}")
                pr = rps.tile([128, qw], F32, tag="r", name=f"pr_{pair}_{qc}")
                for kt in range(ktmax):
                    pt_ps = atps.tile([128, 1024], F32, tag="tps",
                                      name=f"pt_{pair}_{qc}_{kt}")
                    for hloc in range(2):
                        # K=64 contraction; row groups 0-1 / 2-3 concurrent;
                        # each head's T block in its own PSUM bank
                        nc.tensor.matmul(
                            pt_ps[:, 512 * hloc:512 * hloc + qw],
                            lhsT=kt_[64 * hloc:64 * hloc + 64, pair,
                                     kt * 128:kt * 128 + 128],
                            rhs=qt[64 * hloc:64 * hloc + 64, pair, qs:qs + qw],
                            start=True, stop=True,
                            tile_position=(64 * hloc, 0),
                            skip_group_check=True)
                    pt_sb = ptp.tile([128, 1024], BF16, tag="p",
                                     name=f"ptsb_{pair}_{qc}_{kt}")
                    if qw == 512:
                        nc.scalar.activation(pt_sb[:], pt_ps[:],
                                             mybir.ActivationFunctionType.Exp,
                                             scale=scale)
                    else:
                        for hloc in range(2):
                            nc.scalar.activation(
                                pt_sb[:, 512 * hloc:512 * hloc + qw],
                                pt_ps[:, 512 * hloc:512 * hloc + qw],
                                mybir.ActivationFunctionType.Exp, scale=scale)
                    off = kt * 128 - qs
                    if off >= 0:
                        m = off // 128
                        for hloc in range(2):
                            nc.vector.tensor_mul(
                                pt_sb[:, 512 * hloc:512 * hloc + qw],
                                pt_sb[:, 512 * hloc:512 * hloc + qw],
                                mks[:, m * qw:m * qw + qw])
                    for hloc in range(2):
                        h = 2 * pair + hloc
                        nc.tensor.matmul(
                            po[64 * hloc:64 * hloc + 64, :],
                            lhsT=vt[:, kt, 64 * h:64 * h + 64],
                            rhs=pt_sb[:, 512 * hloc:512 * hloc + qw],
                            start=(kt == 0), stop=(kt == ktmax - 1),
                            tile_position=(0, 64 * hloc),
                            skip_group_check=True)
                    for hloc in range(2):
                        nc.tensor.matmul(
                            pr[64 * hloc:64 * hloc + 64, :],
                            lhsT=ones[:, 0:64],
                            rhs=pt_sb[:, 512 * hloc:512 * hloc + qw],
                            start=(kt == 0), stop=(kt == ktmax - 1),
                            tile_position=(0, 64 * hloc),
                            skip_group_check=True)
                ri = rip.tile([128, 512], F32, tag="ri",
                              name=f"ri_{pair}_{qc}")
                nc.vector.reciprocal_approx_fast(out=ri[:, 0:qw], in_=pr[:])
                nc.vector.tensor_mul(at[:, pair, qs:qs + qw], po[:],
                                     ri[:, 0:qw])
    # ---- output projection (separate phase; attention keeps 8 psum banks)
    with tc.tile_pool(name="out_ps", bufs=2, space="PSUM") as outps:
        for qtl in range(n_lt):
            pout = outps.tile([128, 1024], F32, tag="outps",
                              name=f"pout_{qtl}")
            for ct in range(2):
                for eh in range(2):
                    nc.tensor.matmul(
                        pout[:, eh * 512:eh * 512 + 512],
                        lhsT=at[:, ct, qtl * 128:qtl * 128 + 128],
                        rhs=wo[:, ct, eh * 512:eh * 512 + 512],
                        start=(ct == 0), stop=(ct == 1),
                        skip_group_check=True)
            stg = osp.tile([128, 1024], F32, tag="stg", name=f"stg_{qtl}")
            nc.vector.tensor_copy(stg[:, 0:512], pout[:, 0:512])
            nc.scalar.copy(stg[:, 512:1024], pout[:, 512:1024])
            nc.sync.dma_start(out=out_d[qtl * 128:qtl * 128 + 128, :],
                              in_=stg[:])


def build_nc(L=2048):
    """Build + compile the per-core Bass program (same NEFF on all 8 cores)."""
    assert L % 256 == 0
    nc = bacc.Bacc("TRN2", target_bir_lowering=False, debug=False,
                   num_devices=N_CORES)
    with tile.TileContext(nc) as tc:
        with tc.tile_pool(name="persist", bufs=1) as pp, \
             tc.tile_pool(name="ropet", bufs=6) as rtp, \
             tc.tile_pool(name="ptp", bufs=3) as ptp, \
             tc.tile_pool(name="rinvp", bufs=2) as rip, \
             tc.tile_pool(name="ostg", bufs=3) as osp:
            _body(nc, tc, L, pp, rtp, ptp, rip, osp)
    nc.compile()
    return nc


_NC_CACHE = {}


def _get_nc(L):
    if L not in _NC_CACHE:
        _NC_CACHE[L] = build_nc(L)
    return _NC_CACHE[L]


def make_inputs(x, token_positions, Wq, Wk, Wv, Wo):
    """Host-side shard/layout prep -> list of 8 per-core input dicts."""
    B, L, _ = x.shape
    pos = np.asarray(token_positions).astype(np.float64)
    S = ROPE_THETA ** (-2.0 / D_HEAD)
    thetas = S ** np.arange(HALF, dtype=np.float64)
    ang = pos[:, None] * thetas[None, :]          # [L, 32]
    cosL = np.cos(ang).T                          # [32, L]
    sinL = np.sin(ang).T
    # per-channel tables on the natural (head, dim) layout:
    # row p (within a 64-row head block): pair i = (p%64)//2
    # cosb[p] = cos(theta_i * pos); ssin[p] = -sin if dim even else +sin
    cosb = np.empty((128, L), dtype=np.float64)
    ssin = np.empty((128, L), dtype=np.float64)
    for p in range(128):
        i = (p % 64) // 2
        cosb[p] = cosL[i]
        ssin[p] = -sinL[i] if (p % 2 == 0) else sinL[i]
    cosb = cosb.astype(np.float32)
    ssin = ssin.astype(np.float32)

    qw = min(512, L)
    r = np.arange(128)[:, None]
    col = np.arange(qw)[None, :]
    masks = np.concatenate(
        [(col >= r + 128 * m) for m in range(qw // 128)],
        axis=1).astype(NPBF16)  # [128, (qw//128)*qw]

    xts = [np.ascontiguousarray(x[b].astype(NPBF16).T) for b in range(B)]
    in_maps = []
    shard_cache = {}
    for core in range(N_CORES):
        b, hg = core // 4, core % 4
        if hg not in shard_cache:
            rows = slice(hg * 256, hg * 256 + 256)
            shard_cache[hg] = {
                "wqt": np.ascontiguousarray(Wq[rows].astype(NPBF16).T),
                "wkt": np.ascontiguousarray(Wk[rows].astype(NPBF16).T),
                "wvt": np.ascontiguousarray(Wv[rows].astype(NPBF16).T),
                "wot": np.ascontiguousarray(Wo[:, rows].astype(NPBF16).T),
            }
        m = dict(shard_cache[hg])
        m["xt"] = xts[b]
        m["cosb"] = cosb
        m["ssin"] = ssin
        m["masks"] = masks
        in_maps.append(m)
    return in_maps


def kernel(x, token_positions, Wq, Wk, Wv, Wo):
    x = np.asarray(x); Wq = np.asarray(Wq); Wk = np.asarray(Wk)
    Wv = np.asarray(Wv); Wo = np.asarray(Wo)
    B, L, _ = x.shape
    nc = _get_nc(L)
    in_maps = make_inputs(x, token_positions, Wq, Wk, Wv, Wo)
    res = run_bass_kernel_spmd(nc, in_maps, core_ids=list(range(N_CORES)))
    out = np.zeros((B, L, D_MODEL), dtype=np.float32)
    for core in range(N_CORES):
        out[core // 4] += res.results[core]["out"]
    return out


# revision 23
# speedup vs baseline: 1.1002x; 1.0858x over previous
"""Multi-head self-attention (RoPE, causal) Trainium2 Bass kernel, 8 NeuronCores.

Sharding: data-parallel over batch (B=2) x tensor-parallel over heads
(16 heads -> 4 groups of 4). Core c handles batch b=c//4, heads 4*(c%4)..4*(c%4)+3.
Each core computes its 4 heads' attention plus a partial output projection;
the host sums the 4 partial outputs per batch element.

Device-side layout (per core):
  x^T [1024d, L] bf16 (host pre-transposed). Q^T/K^T [256c, L] = W_slice @ x^T.
  RoPE applied in-place on the [channel, L] layout with a partition pair-swap
  (DVE stream_shuffle) + cos / signed-sin tables: 4 DVE ops per tile.
  Attention in transposed [k, q] layout: T = K^T.T @ Q^T (K=64 contraction per
  head), P^T = exp(T/8) (no max subtraction; scores are O(1) by construction),
  O'^T and softmax denominators (ones-matmul) accumulated in PSUM over k tiles.
  Causal: above-diagonal k-tiles skipped entirely, diagonal ones masked.
  Row-group-tiled matmuls must land in distinct PSUM banks, and every open
  PSUM accumulation group owns its whole (partition-range x bank) zero-region.
  Output projection per q chunk from A^T tiles (stationary) x Wo^T slices;
  host sums the 4 partial projections per batch element.
"""
import sys, math

sys.path.insert(0, "/opt/trn_rl_repo")

import numpy as np
import ml_dtypes

import concourse.bacc as bacc
import concourse.bass as bass
import concourse.mybir as mybir
import concourse.tile as tile
from concourse.bass_utils import run_bass_kernel_spmd

BF16 = mybir.dt.bfloat16
F32 = mybir.dt.float32
NPBF16 = ml_dtypes.bfloat16

D_MODEL = 1024
D_HEAD = 64
HALF = D_HEAD // 2
ROPE_THETA = 10000.0
N_CORES = 8
C = 256  # channels per core (4 heads x 64)
SWAP32 = [i ^ 1 for i in range(32)]


def _body(nc, tc, L, pp, rtp, ptp, rip, osp):
    n_lt = L // 128
    n_qk = max(1, L // 512)
    qkw = min(512, L)
    qw = min(512, L)
    n_qch = L // qw
    n_msk = qw // 128

    xt_d = nc.dram_tensor("xt", [D_MODEL, L], BF16, kind="ExternalInput").ap()
    wq_d = nc.dram_tensor("wqt", [D_MODEL, C], BF16, kind="ExternalInput").ap()
    wk_d = nc.dram_tensor("wkt", [D_MODEL, C], BF16, kind="ExternalInput").ap()
    wv_d = nc.dram_tensor("wvt", [D_MODEL, C], BF16, kind="ExternalInput").ap()
    wo_d = nc.dram_tensor("wot", [C, D_MODEL], BF16, kind="ExternalInput").ap()
    cos_d = nc.dram_tensor("cosb", [128, L], F32, kind="ExternalInput").ap()
    sin_d = nc.dram_tensor("ssin", [128, L], F32, kind="ExternalInput").ap()
    mk_d = nc.dram_tensor("masks", [128, n_msk * qw], BF16,
                          kind="ExternalInput").ap()
    out_d = nc.dram_tensor("out", [L, D_MODEL], F32, kind="ExternalOutput").ap()

    # ---- persistent SBUF tensors
    wq = pp.tile([128, 8, C], BF16)
    wk = pp.tile([128, 8, C], BF16)
    wv = pp.tile([128, 8, C], BF16)
    wo = pp.tile([128, 2, D_MODEL], BF16)
    cs = pp.tile([128, L], F32)
    sn = pp.tile([128, L], F32)
    mks = pp.tile([128, n_msk * qw], BF16)
    ones = pp.tile([128, 64], BF16)
    qt = pp.tile([128, 2, L], BF16)
    kt_ = pp.tile([128, 2, L], BF16)
    vt = pp.tile([128, n_lt, C], BF16)
    at = pp.tile([128, 2, L], BF16)
    xts = [pp.tile([128, L], BF16, name=f"xt{i}") for i in range(8)]

    # ---- loads (small tensors first; x^T split per d-tile for pipelining)
    nc.sync.dma_start(out=wq[:], in_=wq_d.rearrange("(a p) c -> p a c", p=128))
    nc.sync.dma_start(out=wk[:], in_=wk_d.rearrange("(a p) c -> p a c", p=128))
    nc.sync.dma_start(out=cs[:], in_=cos_d)
    nc.sync.dma_start(out=sn[:], in_=sin_d)
    nc.sync.dma_start(out=wv[:], in_=wv_d.rearrange("(a p) c -> p a c", p=128))
    for i in range(8):
        nc.sync.dma_start(out=xts[i][:], in_=xt_d[i * 128:(i + 1) * 128, :])
    nc.sync.dma_start(out=wo[:], in_=wo_d.rearrange("(a p) e -> p a e", p=128))
    nc.sync.dma_start(out=mks[:], in_=mk_d)
    nc.gpsimd.memset(ones[:], 1.0)

    # ---- Q^T / K^T projection + RoPE (shuffle pair-swap + cos/signed-sin)
    with tc.tile_pool(name="qk_ps", bufs=6, space="PSUM") as qkps, \
         tc.tile_pool(name="v_ps", bufs=2, space="PSUM") as vps:
        for qc in range(n_qk):
            ls = qc * qkw
            ps = {}
            for nm, w in (("q", wq), ("k", wk)):
                for ct in (0, 1):
                    p = qkps.tile([128, qkw], F32, tag="qkps",
                                  name=f"ps_{nm}{ct}_{qc}")
                    for dt_ in range(8):
                        nc.tensor.matmul(
                            p[:],
                            lhsT=w[:, dt_, ct * 128:ct * 128 + 128],
                            rhs=xts[dt_][:, ls:ls + qkw],
                            start=(dt_ == 0), stop=(dt_ == 7))
                    ps[(nm, ct)] = p
            for nm, dst in (("q", qt), ("k", kt_)):
                for ct in (0, 1):
                    p = ps[(nm, ct)]
                    sh = rtp.tile([128, qkw], F32, tag="t",
                                  name=f"sh_{nm}{ct}{qc}")
                    t1 = rtp.tile([128, qkw], F32, tag="t",
                                  name=f"t1_{nm}{ct}{qc}")
                    t2 = rtp.tile([128, qkw], F32, tag="t",
                                  name=f"t2_{nm}{ct}{qc}")
                    nc.vector.stream_shuffle(sh[:], p[:], SWAP32)
                    nc.vector.tensor_mul(t1[:], p[:], cs[:, ls:ls + qkw])
                    nc.vector.tensor_mul(t2[:], sh[:], sn[:, ls:ls + qkw])
                    nc.vector.tensor_add(dst[:, ct, ls:ls + qkw], t1[:], t2[:])
            # V projection for this chunk's L tiles (keeps attention startable)
            for lt in range(ls // 128, (ls + qkw) // 128):
                pv = vps.tile([128, C], F32, tag="vps", name=f"pv_{lt}")
                for dt_ in range(8):
                    nc.tensor.matmul(
                        pv[:],
                        lhsT=xts[dt_][:, lt * 128:lt * 128 + 128],
                        rhs=wv[:, dt_, :],
                        start=(dt_ == 0), stop=(dt_ == 7))
                nc.scalar.copy(vt[:, lt, :], pv[:])


    # ---- attention + interleaved output projection, per 512-wide q chunk
    scale = 1.0 / math.sqrt(D_HEAD)
    with tc.tile_pool(name="att_ps", bufs=2, space="PSUM") as atps, \
         tc.tile_pool(name="o_ps", bufs=2, space="PSUM") as ops_, \
         tc.tile_pool(name="r_ps", bufs=2, space="PSUM") as rps:
        for qc in range(n_qch):
            qs = qc * qw
            ktmax = (qs + qw) // 128
            for pair in range(2):
                po = ops_.tile([128, qw], F32, tag="o", name=f"po_{pair}_{qc}")
                pr = rps.tile([128, qw], F32, tag="r", name=f"pr_{pair}_{qc}")
                for kt in range(ktmax):
                    pt_ps = atps.tile([128, 1024], F32, tag="tps",
                                      name=f"pt_{pair}_{qc}_{kt}")
                    for hloc in range(2):
                        # K=64 contraction; row groups 0-1 / 2-3 concurrent;
                        # each head's T block in its own PSUM bank
                        nc.tensor.matmul(
                            pt_ps[:, 512 * hloc:512 * hloc + qw],
                            lhsT=kt_[64 * hloc:64 * hloc + 64, pair,
                                     kt * 128:kt * 128 + 128],
                            rhs=qt[64 * hloc:64 * hloc + 64, pair, qs:qs + qw],
                            start=True, stop=True,
                            tile_position=(64 * hloc, 0),
                            skip_group_check=True)
                    pt_sb = ptp.tile([128, 1024], BF16, tag="p",
                                     name=f"ptsb_{pair}_{qc}_{kt}")
                    if qw == 512:
                        nc.scalar.activation(pt_sb[:], pt_ps[:],
                                             mybir.ActivationFunctionType.Exp,
                                             scale=scale)
                    else:
                        for hloc in range(2):
                            nc.scalar.activation(
                                pt_sb[:, 512 * hloc:512 * hloc + qw],
                                pt_ps[:, 512 * hloc:512 * hloc + qw],
                                mybir.ActivationFunctionType.Exp, scale=scale)
                    off = kt * 128 - qs
                    if off >= 0:
                        m = off // 128
                        for hloc in range(2):
                            nc.vector.tensor_mul(
                                pt_sb[:, 512 * hloc:512 * hloc + qw],
                                pt_sb[:, 512 * hloc:512 * hloc + qw],
                                mks[:, m * qw:m * qw + qw])
                    for hloc in range(2):
                        h = 2 * pair + hloc
                        nc.tensor.matmul(
                            po[64 * hloc:64 * hloc + 64, :],
                            lhsT=vt[:, kt, 64 * h:64 * h + 64],
                            rhs=pt_sb[:, 512 * hloc:512 * hloc + qw],
                            start=(kt == 0), stop=(kt == ktmax - 1),
                            tile_position=(0, 64 * hloc),
                            skip_group_check=True)
                    for hloc in range(2):
                        nc.tensor.matmul(
                            pr[64 * hloc:64 * hloc + 64, :],
                            lhsT=ones[:, 0:64],
                            rhs=pt_sb[:, 512 * hloc:512 * hloc + qw],
                            start=(kt == 0), stop=(kt == ktmax - 1),
                            tile_position=(0, 64 * hloc),
                            skip_group_check=True)
                ri = rip.tile([128, 512], F32, tag="ri",
                              name=f"ri_{pair}_{qc}")
                nc.vector.reciprocal_approx_fast(out=ri[:, 0:qw], in_=pr[:])
                nc.vector.tensor_mul(at[:, pair, qs:qs + qw], po[:],
                                     ri[:, 0:qw])
    # ---- output projection (separate phase; attention keeps 8 psum banks)
    with tc.tile_pool(name="out_ps", bufs=2, space="PSUM") as outps:
        for qtl in range(n_lt):
            pout = outps.tile([128, 1024], F32, tag="outps",
                              name=f"pout_{qtl}")
            for ct in range(2):
                for eh in range(2):
                    nc.tensor.matmul(
                        pout[:, eh * 512:eh * 512 + 512],
                        lhsT=at[:, ct, qtl * 128:qtl * 128 + 128],
                        rhs=wo[:, ct, eh * 512:eh * 512 + 512],
                        start=(ct == 0), stop=(ct == 1),
                        skip_group_check=True)
            stg = osp.tile([128, 1024], F32, tag="stg", name=f"stg_{qtl}")
            nc.vector.tensor_copy(stg[:, 0:512], pout[:, 0:512])
            nc.scalar.copy(stg[:, 512:1024], pout[:, 512:1024])
            nc.sync.dma_start(out=out_d[qtl * 128:qtl * 128 + 128, :],
                              in_=stg[:])


def build_nc(L=2048):
    """Build + compile the per-core Bass program (same NEFF on all 8 cores)."""
    assert L % 256 == 0
    nc = bacc.Bacc("TRN2", target_bir_lowering=False, debug=False,
                   num_devices=N_CORES)
    with tile.TileContext(nc) as tc:
        with tc.tile_pool(name="persist", bufs=1) as pp, \
             tc.tile_pool(name="ropet", bufs=6) as rtp, \
             tc.tile_pool(name="ptp", bufs=3) as ptp, \
             tc.tile_pool(name="rinvp", bufs=2) as rip, \
             tc.tile_pool(name="ostg", bufs=3) as osp:
            _body(nc, tc, L, pp, rtp, ptp, rip, osp)
    nc.compile()
    return nc


_NC_CACHE = {}


def _get_nc(L):
    if L not in _NC_CACHE:
        _NC_CACHE[L] = build_nc(L)
    return _NC_CACHE[L]


def make_inputs(x, token_positions, Wq, Wk, Wv, Wo):
    """Host-side shard/layout prep -> list of 8 per-core input dicts."""
    B, L, _ = x.shape
    pos = np.asarray(token_positions).astype(np.float64)
    S = ROPE_THETA ** (-2.0 / D_HEAD)
    thetas = S ** np.arange(HALF, dtype=np.float64)
    ang = pos[:, None] * thetas[None, :]          # [L, 32]
    cosL = np.cos(ang).T                          # [32, L]
    sinL = np.sin(ang).T
    # per-channel tables on the natural (head, dim) layout:
    # row p (within a 64-row head block): pair i = (p%64)//2
    # cosb[p] = cos(theta_i * pos); ssin[p] = -sin if dim even else +sin
    cosb = np.empty((128, L), dtype=np.float64)
    ssin = np.empty((128, L), dtype=np.float64)
    for p in range(128):
        i = (p % 64) // 2
        cosb[p] = cosL[i]
        ssin[p] = -sinL[i] if (p % 2 == 0) else sinL[i]
    cosb = cosb.astype(np.float32)
    ssin = ssin.astype(np.float32)

    qw = min(512, L)
    r = np.arange(128)[:, None]
    col = np.arange(qw)[None, :]
    masks = np.concatenate(
        [(col >= r + 128 * m) for m in range(qw // 128)],
        axis=1).astype(NPBF16)  # [128, (qw//128)*qw]

    xts = [np.ascontiguousarray(x[b].astype(NPBF16).T) for b in range(B)]
    in_maps = []
    shard_cache = {}
    for core in range(N_CORES):
        b, hg = core // 4, core % 4
        if hg not in shard_cache:
            rows = slice(hg * 256, hg * 256 + 256)
            shard_cache[hg] = {
                "wqt": np.ascontiguousarray(Wq[rows].astype(NPBF16).T),
                "wkt": np.ascontiguousarray(Wk[rows].astype(NPBF16).T),
                "wvt": np.ascontiguousarray(Wv[rows].astype(NPBF16).T),
                "wot": np.ascontiguousarray(Wo[:, rows].astype(NPBF16).T),
            }
        m = dict(shard_cache[hg])
        m["xt"] = xts[b]
        m["cosb"] = cosb
        m["ssin"] = ssin
        m["masks"] = masks
        in_maps.append(m)
    return in_maps


def kernel(x, token_positions, Wq, Wk, Wv, Wo):
    x = np.asarray(x); Wq = np.asarray(Wq); Wk = np.asarray(Wk)
    Wv = np.asarray(Wv); Wo = np.asarray(Wo)
    B, L, _ = x.shape
    nc = _get_nc(L)
    in_maps = make_inputs(x, token_positions, Wq, Wk, Wv, Wo)
    res = run_bass_kernel_spmd(nc, in_maps, core_ids=list(range(N_CORES)))
    out = np.zeros((B, L, D_MODEL), dtype=np.float32)
    for core in range(N_CORES):
        out[core // 4] += res.results[core]["out"]
    return out
